# revision 4
# baseline (speedup 1.0000x reference)
"""Trainium2 Bass kernel for nn_Cate1Classifier (SWEM title/desc pooling +
FC + BatchNorm(train) + ReLU + classifier), data-parallel over 8 NeuronCores.

Contract: kernel(**inputs) takes the FULL unsharded inputs (as produced by
setup_inputs()) and returns the FULL [1024, 10] float32 output.

Design notes (v2 — gather-batched, bf16):
- Batch (1024) is sharded 128/core across 8 cores.
- Each core gathers at most 128*250 = 32000 <= 32768 embedding rows, so the
  host builds a PER-CORE compacted bf16 table emb_loc = emb[unique_tokens]
  (padded to a static [32768, 512]) and remaps tokens to int16 local ids.
  This enables InstDMAGatherAnt (nc.gpsimd.dma_gather): ONE Pool-engine
  instruction per 25-position chunk (3200 descriptors) instead of 25
  per-position indirect DMAs — amortizing the ~1us/instruction SWDGE
  descriptor-generation cost ~640x. bf16 rows (1KB) halve HBM traffic vs
  f32 while staying >= 512B (no small-descriptor penalty).
- dma_gather layout (transpose=False): linear index j -> dst[j%128, j//128];
  j%128 is the sample partition. Index tile is [128, nidx/16] int16 with
  idx_j at [j%16, j//16], replicated 8x across partition groups (one copy
  per Q7 core). single_packet=False — True hangs the device at this size.
- Padding is handled index-side: padded slots gather a duplicate of the
  sample's token 0. Max-pool is then exact with no masking; the sum-pool is
  fixed up per chunk with one diag(-npad_chunk) @ tok0_row matmul (npad are
  small integers — exact in bf16, so the cancellation is clean).
- Sum-pool rides the otherwise-idle PE as bf16 identity-copy matmuls
  accumulating in f32 PSUM; max-pool is a bf16 tensor_tensor tree on DVE
  (2-byte contiguous operands hit the DVE 2x mode).
- Pooled features are PE-transposed so the FC (all bf16) produces h^T
  (hidden-on-partitions); BatchNorm scale/shift become per-partition
  scalars applied by the ACT engine fused with ReLU.
- BatchNorm uses full-batch statistics: per-core sum(h), sum(h^2) are
  AllReduce'd across the 8 cores (8KB payload).
- b_fc is omitted: BN immediately follows the FC, so a constant column
  shift cancels exactly in (h - mean).
- W_fc (bf16, 4MB) is streamed via the SWDGE (gpsimd) queue AFTER the
  gather instructions so its transfers don't delay the gather-bound phase.
"""

import sys

for _p in ("/opt/trn_rl_repo", "/root/.axon_site/_ro/trn_rl_repo"):
    if _p not in sys.path:
        sys.path.insert(0, _p)

import numpy as np
import ml_dtypes

from concourse import bass, bacc, tile, mybir
from concourse import bass_utils

# Problem shape (hardcoded per the task contract).
B, LT, LD = 1024, 50, 200
V, D = 100000, 512
H, C = 1024, 10
N_CORES = 8
PB = B // N_CORES  # 128 samples per core
KC = 25  # token positions per gather/reduce chunk
NCHUNKS = (LT + LD) // KC  # 2 title + 8 desc
VLOC = 32768  # per-core compacted vocab (>= 128*250 worst case = 32000)
BN_EPS = 1e-5

F32 = mybir.dt.float32
BF16 = mybir.dt.bfloat16
I16 = mybir.dt.int16
AF = mybir.ActivationFunctionType
OP = mybir.AluOpType
NPBF = ml_dtypes.bfloat16

_PROGRAM = None


def _tree_reduce(nc, g, s, acc, op, first_chunk):
    """Reduce the 25 [128, D] slices of chunk tile g with `op` into acc.

    First level folds into scratch s so g is preserved (the PE sum-matmuls
    read g concurrently).
    """
    ts = nc.vector.tensor_tensor
    ts(out=s[:, 0:12 * D], in0=g[:, 0:12 * D], in1=g[:, 12 * D:24 * D], op=op)
    for a, b, n in ((0, 6, 6), (0, 3, 3), (1, 2, 1), (0, 1, 1)):
        ts(out=s[:, a * D:(a + n) * D], in0=s[:, a * D:(a + n) * D],
           in1=s[:, b * D:(b + n) * D], op=op)
    if first_chunk:
        ts(out=acc[:], in0=s[:, 0:D], in1=g[:, 24 * D:25 * D], op=op)
    else:
        ts(out=s[:, 0:D], in0=s[:, 0:D], in1=g[:, 24 * D:25 * D], op=op)
        ts(out=acc[:], in0=acc[:], in1=s[:, 0:D], op=op)


def _build():
    nc = bacc.Bacc("TRN2", target_bir_lowering=False, debug=False,
                   num_devices=N_CORES)

    NIDX = 128 * KC  # indices per gather chunk
    WC = NIDX // 16  # wrapped-index columns per chunk

    emb_loc = nc.dram_tensor("emb_loc", [VLOC, D], BF16, kind="ExternalInput")
    t_widx = nc.dram_tensor("t_widx", [128, (LT // KC) * WC], I16,
                            kind="ExternalInput")
    d_widx = nc.dram_tensor("d_widx", [128, (LD // KC) * WC], I16,
                            kind="ExternalInput")
    scal = nc.dram_tensor("scal", [PB, 4], F32, kind="ExternalInput")
    wfc = nc.dram_tensor("wfc", [4 * D, H], BF16, kind="ExternalInput")
    wclf = nc.dram_tensor("wclf", [H, C], BF16, kind="ExternalInput")
    bclf = nc.dram_tensor("bclf", [1, C], BF16, kind="ExternalInput")
    gamma_t = nc.dram_tensor("gamma_t", [128, 8], F32, kind="ExternalInput")
    beta_t = nc.dram_tensor("beta_t", [128, 8], F32, kind="ExternalInput")
    ident = nc.dram_tensor("ident", [128, 128], BF16, kind="ExternalInput")
    ones1 = nc.dram_tensor("ones1", [1, 128], BF16, kind="ExternalInput")
    # per-chunk diag(-npad_chunk) matrices (title 2 + desc 8, stacked)
    dnpad = nc.dram_tensor("dnpad", [NCHUNKS * 128, 128], BF16,
                           kind="ExternalInput")
    logits = nc.dram_tensor("logits", [PB, C], F32, kind="ExternalOutput")

    with tile.TileContext(nc) as tc:
        with tc.tile_pool(name="const", bufs=1) as cp, \
             tc.tile_pool(name="gpool", bufs=3) as gp, \
             tc.tile_pool(name="spool", bufs=1) as sp, \
             tc.tile_pool(name="wpool", bufs=16) as wp, \
             tc.tile_pool(name="psA", bufs=2, space="PSUM") as psA, \
             tc.tile_pool(name="psB", bufs=1, space="PSUM") as psB, \
             tc.tile_pool(name="psS", bufs=1, space="PSUM") as psS, \
             tc.tile_pool(name="dram", bufs=1, space="DRAM") as dp:

            # --- constant loads ---
            t_idx_t = cp.tile([128, (LT // KC) * WC], I16, tag="tidx")
            d_idx_t = cp.tile([128, (LD // KC) * WC], I16, tag="didx")
            scal_t = cp.tile([PB, 4], F32, tag="scal")
            gam_t = cp.tile([128, 8], F32, tag="gam")
            bet_t = cp.tile([128, 8], F32, tag="bet")
            id_t = cp.tile([128, 128], BF16, tag="ident")
            on_t = cp.tile([1, 128], BF16, tag="ones1")
            bc_t = cp.tile([1, C], BF16, tag="bclf")
            for dst, src in ((t_idx_t, t_widx), (d_idx_t, d_widx),
                             (scal_t, scal), (gam_t, gamma_t),
                             (bet_t, beta_t), (id_t, ident),
                             (on_t, ones1), (bc_t, bclf)):
                nc.sync.dma_start(dst[:], src[:])
            wclf_t = []
            for mb in range(8):
                w = cp.tile([128, C], BF16, tag=f"wclf{mb}")
                nc.sync.dma_start(w[:], wclf[mb * 128:(mb + 1) * 128, :])
                wclf_t.append(w)
            dnp_t = []
            for i in range(NCHUNKS):
                dt_ = cp.tile([128, 128], BF16, tag=f"dnp{i}", name=f"dnp{i}")
                nc.sync.dma_start(dt_[:], dnpad[i * 128:(i + 1) * 128, :])
                dnp_t.append(dt_)

            # --- pooling: acc tiles + gather/reduce chunks ---
            accs = {}
            chunk_base = {"t": 0, "d": LT // KC}
            for fld, idx_t, L, inv_col in (
                    ("t", t_idx_t, LT, 0), ("d", d_idx_t, LD, 1)):
                acc_s = cp.tile([PB, D], BF16, tag=f"acc_s{fld}",
                                name=f"acc_s{fld}")
                acc_m = cp.tile([PB, D], BF16, tag=f"acc_m{fld}",
                                name=f"acc_m{fld}")
                sav = cp.tile([PB, D], BF16, tag=f"sav{fld}", name=f"sav{fld}")
                ps_s = psS.tile([128, D], F32, tag=f"ps_s{fld}",
                                name=f"ps_s{fld}")
                accs[fld] = (acc_s, acc_m)
                nchunks = L // KC
                for c in range(nchunks):
                    g = gp.tile([PB, KC * D], BF16, tag="g")
                    nc.gpsimd.dma_gather(
                        g[:].rearrange("p (k d) -> p k d", k=KC),
                        emb_loc[:],
                        idx_t[:, c * WC:(c + 1) * WC],
                        NIDX,
                        NIDX,
                        D,
                        single_packet=False,
                    )
                    if c == 0:
                        nc.vector.tensor_copy(sav[:], g[:, 0:D])
                    # sum-pool on the (otherwise idle) PE: psum += I.T @ e_p
                    for j in range(KC):
                        nc.tensor.matmul(ps_s[:], lhsT=id_t[:],
                                         rhs=g[:, j * D:(j + 1) * D],
                                         start=(c == 0 and j == 0), stop=False)
                    # cancel this chunk's padding (padded slots duplicate
                    # token 0): psum += diag(-npad_chunk) @ e_tok0. Keeping
                    # this per-chunk bounds the f32 partial-sum magnitude.
                    nc.tensor.matmul(ps_s[:], lhsT=dnp_t[chunk_base[fld] + c][:],
                                     rhs=sav[:],
                                     start=False, stop=(c == nchunks - 1))
                    # max-pool tree on DVE (bf16 2x mode)
                    s = sp.tile([PB, 12 * D], BF16, tag="scr")
                    _tree_reduce(nc, g, s, acc_m, OP.max, c == 0)
                # avg = psum_sum / len
                nc.vector.tensor_scalar_mul(
                    acc_s[:], ps_s[:], scal_t[:, inv_col:inv_col + 1])

            # --- FC weights: stream AFTER the gathers on the SWDGE queue so
            # their transfers sort behind the gather transfers ---
            wfc_t = []
            for kc in range(16):
                w = wp.tile([128, H], BF16, tag="wfc")
                nc.gpsimd.dma_start(w[:], wfc[kc * 128:(kc + 1) * 128, :])
                wfc_t.append(w)

            # --- transpose pooled features: swem^T, 16 [128,128] tiles ---
            # swem column order: [t_avg | t_max | d_avg | d_max]
            order = [accs["t"][0], accs["t"][1], accs["d"][0], accs["d"][1]]
            swemT = []
            for i in range(16):
                src = order[i // 4]
                blk = i % 4
                pt = psA.tile([128, 128], BF16, tag="tps")
                nc.tensor.transpose(pt[:], src[:, blk * 128:(blk + 1) * 128],
                                    id_t[:])
                st = cp.tile([128, 128], BF16, tag=f"swemT{i}")
                # split PSUM->SBUF bf16 copies between DVE and ACT
                if i % 2 == 0:
                    nc.vector.tensor_copy(st[:], pt[:])
                else:
                    nc.scalar.copy(st[:], pt[:])
                swemT.append(st)

            # --- FC: h^T[mb] [128 hidden, 128 samples], mb in 0..7 ---
            h_ps = [psB.tile([128, 512], F32, tag="hps0", name="hps0"),
                    psB.tile([128, 512], F32, tag="hps1", name="hps1")]
            # PSUM `start` clears the has_written bits for the WHOLE bank, so
            # emit start only on the first matmul touching each bank (the
            # other slices then overwrite-on-first-touch per element), and
            # stop only on the last matmul into that bank.
            for kc in range(16):
                for mb in range(8):
                    nc.tensor.matmul(
                        h_ps[mb // 4][:, (mb % 4) * 128:(mb % 4 + 1) * 128],
                        lhsT=wfc_t[kc][:, mb * 128:(mb + 1) * 128],
                        rhs=swemT[kc][:],
                        start=(kc == 0 and mb % 4 == 0),
                        stop=(kc == 15 and mb % 4 == 3))

            # --- batch stats: s1 = sum_n h, s2 = sum_n h^2 (per hidden) ---
            # read h straight from PSUM; square on the idle ACT engine
            s12 = cp.tile([128, 16], F32, tag="s12")
            for mb in range(8):
                hps = h_ps[mb // 4][:, (mb % 4) * 128:(mb % 4 + 1) * 128]
                sq = sp.tile([128, 128], F32, tag="sq")
                nc.scalar.activation(sq[:], hps, AF.Square)
                nc.vector.reduce_sum(s12[:, mb:mb + 1], hps,
                                     axis=mybir.AxisListType.X)
                nc.vector.reduce_sum(s12[:, 8 + mb:9 + mb], sq[:],
                                     axis=mybir.AxisListType.X)

            # --- AllReduce batch stats across the 8 cores ---
            cc_in = dp.tile([128, 16], F32, tag="ccin")
            cc_out = dp.tile([128, 16], F32, tag="ccout")
            nc.sync.dma_start(cc_in[:], s12[:])
            nc.gpsimd.collective_compute(
                "AllReduce", OP.add,
                replica_groups=[list(range(N_CORES))],
                ins=[cc_in.opt()], outs=[cc_out.opt()],
            )
            s12g = cp.tile([128, 16], F32, tag="s12g")
            nc.sync.dma_start(s12g[:], cc_out[:])

            # --- BN scale/shift (per hidden unit, [128, 8]) ---
            mean = cp.tile([128, 8], F32, tag="mean")
            var = cp.tile([128, 8], F32, tag="var")
            scale = cp.tile([128, 8], F32, tag="scale")
            shift = cp.tile([128, 8], F32, tag="shift")
            inv_b = 1.0 / float(B)
            nc.vector.tensor_scalar_mul(mean[:], s12g[:, 0:8], inv_b)
            nc.vector.tensor_mul(scale[:], mean[:], mean[:])     # mean^2 (tmp)
            nc.vector.tensor_scalar_mul(var[:], s12g[:, 8:16], inv_b)
            nc.vector.tensor_sub(var[:], var[:], scale[:])       # var -= mean^2
            nc.vector.tensor_scalar_add(var[:], var[:], BN_EPS)
            nc.scalar.activation(var[:], var[:], AF.Sqrt)        # var = std
            nc.vector.reciprocal(scale[:], var[:])               # scale = 1/std
            nc.vector.tensor_mul(scale[:], scale[:], gam_t[:])   # *= gamma
            nc.vector.tensor_mul(shift[:], mean[:], scale[:])
            nc.vector.tensor_sub(shift[:], bet_t[:], shift[:])   # beta - m*s

            # --- BN apply + ReLU on ACT, then classifier matmuls ---
            o_ps = psB.tile([128, C], F32, tag="ops")
            for mb in range(8):
                r = cp.tile([128, 128], BF16, tag=f"rT{mb}", name=f"rT{mb}")
                nc.scalar.activation(
                    r[:], h_ps[mb // 4][:, (mb % 4) * 128:(mb % 4 + 1) * 128],
                    AF.Relu,
                    bias=shift[:, mb:mb + 1],
                    scale=scale[:, mb:mb + 1])
                nc.tensor.matmul(o_ps[:], lhsT=r[:], rhs=wclf_t[mb][:],
                                 start=(mb == 0), stop=False)
            # + b_clf via rank-1 ones matmul
            nc.tensor.matmul(o_ps[:], lhsT=on_t[:], rhs=bc_t[:],
                             start=False, stop=True)
            out_sb = cp.tile([128, C], F32, tag="outsb")
            nc.vector.tensor_copy(out_sb[:], o_ps[:])
            nc.sync.dma_start(logits[:], out_sb[:])

    nc.compile()
    return nc


def _get_program():
    global _PROGRAM
    if _PROGRAM is None:
        _PROGRAM = _build()
    return _PROGRAM


def _wrap_idx(lid):
    """[128, L] int16 local ids -> [128, (L/KC)*WC] wrapped dma_gather index
    tile: per chunk, linear j = k*128 + s reads lid[s, c*KC+k]; idx_j sits at
    [j%16, j//16]; the 16-partition block is replicated 8x."""
    L = lid.shape[1]
    cols = []
    for c in range(L // KC):
        lin = lid[:, c * KC:(c + 1) * KC].T.reshape(-1)  # j = k*128 + s
        cols.append(lin.reshape(-1, 16).T)               # [16, WC]
    wrapped = np.concatenate(cols, axis=1)               # [16, (L/KC)*WC]
    return np.ascontiguousarray(np.tile(wrapped, (8, 1)).astype(np.int16))


def _prep_in_maps(title, desc, t_len, d_len, emb, W_fc, b_fc, gamma, beta,
                  W_clf, b_clf):
    # sanitize: padded positions duplicate token 0 (keeps max exact; sum is
    # corrected on-device with the npad counts)
    def sanitize(tok, lens, L):
        tok = np.asarray(tok)
        valid = np.arange(L)[None, :] < np.asarray(lens)[:, None]
        return np.where(valid, tok, tok[:, :1]).astype(np.int32)

    title_s = sanitize(title, t_len, LT)
    desc_s = sanitize(desc, d_len, LD)
    t_len = np.asarray(t_len).astype(np.float64)
    d_len = np.asarray(d_len).astype(np.float64)
    scal = np.stack([
        1.0 / np.maximum(t_len, 1.0),
        1.0 / np.maximum(d_len, 1.0),
        np.zeros_like(t_len),
        np.zeros_like(d_len),
    ], axis=1).astype(np.float32)  # [B, 4]

    emb_bf = np.asarray(emb, dtype=np.float32).astype(NPBF)
    wfc = np.asarray(W_fc, dtype=np.float32).astype(NPBF)
    wclf = np.asarray(W_clf, dtype=np.float32).astype(NPBF)
    bclf = np.asarray(b_clf, dtype=np.float32).astype(NPBF).reshape(1, C)
    gamma_t = np.ascontiguousarray(
        np.asarray(gamma, dtype=np.float32).reshape(8, 128).T)
    beta_t = np.ascontiguousarray(
        np.asarray(beta, dtype=np.float32).reshape(8, 128).T)
    ident = np.eye(128, dtype=np.float32).astype(NPBF)
    ones1 = np.ones((1, 128), dtype=np.float32).astype(NPBF)

    # diag(-npad_chunk) per (field, chunk): npad_chunk[p] = # padded slots of
    # sample p among positions [c*KC, (c+1)*KC)
    def pad_counts(lens, L):
        cs = []
        for c in range(L // KC):
            a, b = c * KC, (c + 1) * KC
            cs.append(np.clip(b - np.maximum(lens, a), 0, KC))
        return cs  # list of [B]

    npc = pad_counts(t_len, LT) + pad_counts(d_len, LD)
    rng128 = np.arange(128)

    in_maps = []
    for i in range(N_CORES):
        sl = slice(i * PB, (i + 1) * PB)
        # per-core compacted vocab: <= 128*250 = 32000 unique rows, so int16
        # local ids always fit
        tok_all = np.concatenate([title_s[sl], desc_s[sl]], axis=1)  # [128,250]
        uniq, inv = np.unique(tok_all, return_inverse=True)
        lid = inv.reshape(PB, LT + LD).astype(np.int16)
        emb_loc = np.zeros((VLOC, D), dtype=NPBF)
        emb_loc[:len(uniq)] = emb_bf[uniq]
        dn = np.zeros((NCHUNKS * 128, 128), dtype=np.float32)
        for ci, cnt in enumerate(npc):
            dn[ci * 128 + rng128, rng128] = -cnt[sl].astype(np.float32)
        in_maps.append({
            "emb_loc": emb_loc,
            "t_widx": _wrap_idx(lid[:, :LT]),
            "d_widx": _wrap_idx(lid[:, LT:]),
            "scal": np.ascontiguousarray(scal[sl]),
            "wfc": wfc, "wclf": wclf, "bclf": bclf,
            "gamma_t": gamma_t, "beta_t": beta_t,
            "ident": ident, "ones1": ones1,
            "dnpad": dn.astype(NPBF),
        })
    return in_maps


def kernel(title, desc, t_len, d_len, emb, W_fc, b_fc, gamma, beta,
           W_clf, b_clf):
    nc = _get_program()
    in_maps = _prep_in_maps(title, desc, t_len, d_len, emb, W_fc, b_fc,
                            gamma, beta, W_clf, b_clf)
    res = bass_utils.run_bass_kernel_spmd(nc, in_maps,
                                          core_ids=list(range(N_CORES)))
    return np.concatenate([res.results[i]["logits"] for i in range(N_CORES)],
                          axis=0)


# revision 5
# speedup vs baseline: 1.0675x; 1.0675x over previous
"""Trainium2 Bass kernel for nn_Cate1Classifier (SWEM title/desc pooling +
FC + BatchNorm(train) + ReLU + classifier), data-parallel over 8 NeuronCores.

Contract: kernel(**inputs) takes the FULL unsharded inputs (as produced by
setup_inputs()) and returns the FULL [1024, 10] float32 output.

Design notes (v2 — gather-batched, bf16):
- Batch (1024) is sharded 128/core across 8 cores.
- Each core gathers at most 128*250 = 32000 <= 32768 embedding rows, so the
  host builds a PER-CORE compacted bf16 table emb_loc = emb[unique_tokens]
  (padded to a static [32768, 512]) and remaps tokens to int16 local ids.
  This enables InstDMAGatherAnt (nc.gpsimd.dma_gather): ONE Pool-engine
  instruction per 25-position chunk (3200 descriptors) instead of 25
  per-position indirect DMAs — amortizing the ~1us/instruction SWDGE
  descriptor-generation cost ~640x. bf16 rows (1KB) halve HBM traffic vs
  f32 while staying >= 512B (no small-descriptor penalty).
- dma_gather layout (transpose=False): linear index j -> dst[j%128, j//128];
  j%128 is the sample partition. Index tile is [128, nidx/16] int16 with
  idx_j at [j%16, j//16], replicated 8x across partition groups (one copy
  per Q7 core). single_packet=False — True hangs the device at this size.
- Padding is handled index-side: padded slots gather a duplicate of the
  sample's token 0. Max-pool is then exact with no masking; the sum-pool is
  fixed up per chunk with one diag(-npad_chunk) @ tok0_row matmul (npad are
  small integers — exact in bf16, so the cancellation is clean).
- Sum-pool rides the otherwise-idle PE as bf16 identity-copy matmuls
  accumulating in f32 PSUM; max-pool is a bf16 tensor_tensor tree on DVE
  (2-byte contiguous operands hit the DVE 2x mode).
- Pooled features are PE-transposed so the FC (all bf16) produces h^T
  (hidden-on-partitions); BatchNorm scale/shift become per-partition
  scalars applied by the ACT engine fused with ReLU.
- BatchNorm uses full-batch statistics: per-core sum(h), sum(h^2) are
  AllReduce'd across the 8 cores (8KB payload).
- b_fc is omitted: BN immediately follows the FC, so a constant column
  shift cancels exactly in (h - mean).
- W_fc (bf16, 4MB) is streamed via the SWDGE (gpsimd) queue AFTER the
  gather instructions so its transfers don't delay the gather-bound phase.
"""

import sys

for _p in ("/opt/trn_rl_repo", "/root/.axon_site/_ro/trn_rl_repo"):
    if _p not in sys.path:
        sys.path.insert(0, _p)

import numpy as np
import ml_dtypes

from concourse import bass, bacc, tile, mybir
from concourse import bass_utils

# Problem shape (hardcoded per the task contract).
B, LT, LD = 1024, 50, 200
V, D = 100000, 512
H, C = 1024, 10
N_CORES = 8
PB = B // N_CORES  # 128 samples per core
KC = 25  # token positions per gather/reduce chunk
NCHUNKS = (LT + LD) // KC  # 2 title + 8 desc
VLOC = 32768  # per-core compacted vocab (>= 128*250 worst case = 32000)
BN_EPS = 1e-5

F32 = mybir.dt.float32
BF16 = mybir.dt.bfloat16
I16 = mybir.dt.int16
AF = mybir.ActivationFunctionType
OP = mybir.AluOpType
NPBF = ml_dtypes.bfloat16

_PROGRAM = None


def _tree_reduce(nc, g, s, acc, op, first_chunk):
    """Reduce the 25 [128, D] slices of chunk tile g with `op` into acc.

    First level folds into scratch s so g is preserved (the PE sum-matmuls
    read g concurrently).
    """
    ts = nc.vector.tensor_tensor
    ts(out=s[:, 0:12 * D], in0=g[:, 0:12 * D], in1=g[:, 12 * D:24 * D], op=op)
    for a, b, n in ((0, 6, 6), (0, 3, 3), (1, 2, 1), (0, 1, 1)):
        ts(out=s[:, a * D:(a + n) * D], in0=s[:, a * D:(a + n) * D],
           in1=s[:, b * D:(b + n) * D], op=op)
    if first_chunk:
        ts(out=acc[:], in0=s[:, 0:D], in1=g[:, 24 * D:25 * D], op=op)
    else:
        ts(out=s[:, 0:D], in0=s[:, 0:D], in1=g[:, 24 * D:25 * D], op=op)
        ts(out=acc[:], in0=acc[:], in1=s[:, 0:D], op=op)


def _build():
    nc = bacc.Bacc("TRN2", target_bir_lowering=False, debug=False,
                   num_devices=N_CORES)

    NIDX = 128 * KC  # indices per gather chunk
    WC = NIDX // 16  # wrapped-index columns per chunk

    emb_loc = nc.dram_tensor("emb_loc", [VLOC, D], BF16, kind="ExternalInput")
    t_widx = nc.dram_tensor("t_widx", [128, (LT // KC) * WC], I16,
                            kind="ExternalInput")
    d_widx = nc.dram_tensor("d_widx", [128, (LD // KC) * WC], I16,
                            kind="ExternalInput")
    scal = nc.dram_tensor("scal", [PB, 4], F32, kind="ExternalInput")
    wfc = nc.dram_tensor("wfc", [4 * D, H], BF16, kind="ExternalInput")
    wclf = nc.dram_tensor("wclf", [H, C], BF16, kind="ExternalInput")
    bclf = nc.dram_tensor("bclf", [1, C], BF16, kind="ExternalInput")
    gamma_t = nc.dram_tensor("gamma_t", [128, 8], F32, kind="ExternalInput")
    beta_t = nc.dram_tensor("beta_t", [128, 8], F32, kind="ExternalInput")
    ident = nc.dram_tensor("ident", [128, 128], BF16, kind="ExternalInput")
    ones1 = nc.dram_tensor("ones1", [1, 128], BF16, kind="ExternalInput")
    # per-chunk diag(-npad_chunk) matrices (title 2 + desc 8, stacked)
    dnpad = nc.dram_tensor("dnpad", [NCHUNKS * 128, 128], BF16,
                           kind="ExternalInput")
    logits = nc.dram_tensor("logits", [PB, C], F32, kind="ExternalOutput")

    with tile.TileContext(nc) as tc:
        with tc.tile_pool(name="const", bufs=1) as cp, \
             tc.tile_pool(name="gpool", bufs=3) as gp, \
             tc.tile_pool(name="spool", bufs=1) as sp, \
             tc.tile_pool(name="wpool", bufs=16) as wp, \
             tc.tile_pool(name="psA", bufs=2, space="PSUM") as psA, \
             tc.tile_pool(name="psB", bufs=1, space="PSUM") as psB, \
             tc.tile_pool(name="psS", bufs=1, space="PSUM") as psS, \
             tc.tile_pool(name="dram", bufs=1, space="DRAM") as dp:

            # --- constant loads ---
            t_idx_t = cp.tile([128, (LT // KC) * WC], I16, tag="tidx")
            d_idx_t = cp.tile([128, (LD // KC) * WC], I16, tag="didx")
            scal_t = cp.tile([PB, 4], F32, tag="scal")
            gam_t = cp.tile([128, 8], F32, tag="gam")
            bet_t = cp.tile([128, 8], F32, tag="bet")
            id_t = cp.tile([128, 128], BF16, tag="ident")
            on_t = cp.tile([1, 128], BF16, tag="ones1")
            bc_t = cp.tile([1, C], BF16, tag="bclf")
            for dst, src in ((t_idx_t, t_widx), (d_idx_t, d_widx),
                             (scal_t, scal), (gam_t, gamma_t),
                             (bet_t, beta_t), (id_t, ident),
                             (on_t, ones1), (bc_t, bclf)):
                nc.sync.dma_start(dst[:], src[:])
            wclf_t = []
            for mb in range(8):
                w = cp.tile([128, C], BF16, tag=f"wclf{mb}")
                nc.sync.dma_start(w[:], wclf[mb * 128:(mb + 1) * 128, :])
                wclf_t.append(w)
            dnp_t = []
            for i in range(NCHUNKS):
                dt_ = cp.tile([128, 128], BF16, tag=f"dnp{i}", name=f"dnp{i}")
                nc.sync.dma_start(dt_[:], dnpad[i * 128:(i + 1) * 128, :])
                dnp_t.append(dt_)

            # --- pooling: acc tiles + gather/reduce chunks ---
            accs = {}
            chunk_base = {"t": 0, "d": LT // KC}
            for fld, idx_t, L, inv_col in (
                    ("t", t_idx_t, LT, 0), ("d", d_idx_t, LD, 1)):
                acc_s = cp.tile([PB, D], BF16, tag=f"acc_s{fld}",
                                name=f"acc_s{fld}")
                acc_m = cp.tile([PB, D], BF16, tag=f"acc_m{fld}",
                                name=f"acc_m{fld}")
                sav = cp.tile([PB, D], BF16, tag=f"sav{fld}", name=f"sav{fld}")
                ps_s = psS.tile([128, D], F32, tag=f"ps_s{fld}",
                                name=f"ps_s{fld}")
                accs[fld] = (acc_s, acc_m)
                nchunks = L // KC
                for c in range(nchunks):
                    g = gp.tile([PB, KC * D], BF16, tag="g")
                    nc.gpsimd.dma_gather(
                        g[:].rearrange("p (k d) -> p k d", k=KC),
                        emb_loc[:],
                        idx_t[:, c * WC:(c + 1) * WC],
                        NIDX,
                        NIDX,
                        D,
                        single_packet=False,
                    )
                    if c == 0:
                        nc.vector.tensor_copy(sav[:], g[:, 0:D])
                    # sum-pool on the (otherwise idle) PE: psum += I.T @ e_p
                    for j in range(KC):
                        nc.tensor.matmul(ps_s[:], lhsT=id_t[:],
                                         rhs=g[:, j * D:(j + 1) * D],
                                         start=(c == 0 and j == 0), stop=False)
                    # cancel this chunk's padding (padded slots duplicate
                    # token 0): psum += diag(-npad_chunk) @ e_tok0. Keeping
                    # this per-chunk bounds the f32 partial-sum magnitude.
                    nc.tensor.matmul(ps_s[:], lhsT=dnp_t[chunk_base[fld] + c][:],
                                     rhs=sav[:],
                                     start=False, stop=(c == nchunks - 1))
                    # max-pool tree on DVE (bf16 2x mode)
                    s = sp.tile([PB, 12 * D], BF16, tag="scr")
                    _tree_reduce(nc, g, s, acc_m, OP.max, c == 0)
                # avg = psum_sum / len
                nc.vector.tensor_scalar_mul(
                    acc_s[:], ps_s[:], scal_t[:, inv_col:inv_col + 1])

            # --- FC weights: stream AFTER the gathers on the SWDGE queue so
            # their transfers sort behind the gather transfers ---
            wfc_t = []
            for kc in range(16):
                w = wp.tile([128, H], BF16, tag="wfc")
                nc.gpsimd.dma_start(w[:], wfc[kc * 128:(kc + 1) * 128, :])
                wfc_t.append(w)

            # --- transpose pooled features: swem^T, 16 [128,128] tiles ---
            # swem column order: [t_avg | t_max | d_avg | d_max]
            order = [accs["t"][0], accs["t"][1], accs["d"][0], accs["d"][1]]
            swemT = []
            for i in range(16):
                src = order[i // 4]
                blk = i % 4
                pt = psA.tile([128, 128], BF16, tag="tps")
                nc.tensor.transpose(pt[:], src[:, blk * 128:(blk + 1) * 128],
                                    id_t[:])
                st = cp.tile([128, 128], BF16, tag=f"swemT{i}")
                # split PSUM->SBUF bf16 copies between DVE and ACT
                if i % 2 == 0:
                    nc.vector.tensor_copy(st[:], pt[:])
                else:
                    nc.scalar.copy(st[:], pt[:])
                swemT.append(st)

            # --- FC: h^T[mb] [128 hidden, 128 samples], mb in 0..7 ---
            # bank-outer order so bank 0's stats (ACT/DVE) overlap bank 1's
            # matmuls on PE.
            h_ps = [psB.tile([128, 512], F32, tag="hps0", name="hps0"),
                    psB.tile([128, 512], F32, tag="hps1", name="hps1")]
            s12 = cp.tile([128, 16], F32, tag="s12")
            # PSUM `start` clears the has_written bits for the WHOLE bank, so
            # emit start only on the first matmul touching each bank (the
            # other slices then overwrite-on-first-touch per element), and
            # stop only on the last matmul into that bank.
            for bank in range(2):
                for kc in range(16):
                    for q in range(4):
                        mb = bank * 4 + q
                        nc.tensor.matmul(
                            h_ps[bank][:, q * 128:(q + 1) * 128],
                            lhsT=wfc_t[kc][:, mb * 128:(mb + 1) * 128],
                            rhs=swemT[kc][:],
                            start=(kc == 0 and q == 0),
                            stop=(kc == 15 and q == 3))
                # batch stats: s1 = sum_n h, s2 = sum_n h^2 (per hidden);
                # square on the otherwise-idle ACT engine, read h from PSUM
                for q in range(4):
                    mb = bank * 4 + q
                    hps = h_ps[bank][:, q * 128:(q + 1) * 128]
                    sq = sp.tile([128, 128], F32, tag="sq")
                    nc.scalar.activation(sq[:], hps, AF.Square)
                    nc.vector.reduce_sum(s12[:, mb:mb + 1], hps,
                                         axis=mybir.AxisListType.X)
                    nc.vector.reduce_sum(s12[:, 8 + mb:9 + mb], sq[:],
                                         axis=mybir.AxisListType.X)

            # --- AllGather batch stats across the 8 cores + local sum ---
            # (the cost model charges AllReduce 1.875x the constant overhead;
            # AllGather + 7 local adds is ~13us cheaper)
            cc_in = dp.tile([128, 16], F32, tag="ccin")
            cc_out = dp.tile([N_CORES * 128, 16], F32, tag="ccout")
            nc.sync.dma_start(cc_in[:], s12[:])
            nc.gpsimd.collective_compute(
                "AllGather", OP.bypass,
                replica_groups=[list(range(N_CORES))],
                ins=[cc_in.opt()], outs=[cc_out.opt()],
            )
            cs = cp.tile([128, N_CORES * 16], F32, tag="cs")
            nc.sync.dma_start(
                cs[:].rearrange("p (g c) -> p g c", g=N_CORES),
                cc_out[:].rearrange("(g p) c -> p g c", g=N_CORES))
            s12g = cp.tile([128, 16], F32, tag="s12g")
            nc.vector.tensor_tensor(out=s12g[:], in0=cs[:, 0:16],
                                    in1=cs[:, 16:32], op=OP.add)
            for gblk in range(2, N_CORES):
                nc.vector.tensor_tensor(out=s12g[:], in0=s12g[:],
                                        in1=cs[:, gblk * 16:(gblk + 1) * 16],
                                        op=OP.add)

            # --- BN scale/shift (per hidden unit, [128, 8]) ---
            mean = cp.tile([128, 8], F32, tag="mean")
            var = cp.tile([128, 8], F32, tag="var")
            scale = cp.tile([128, 8], F32, tag="scale")
            shift = cp.tile([128, 8], F32, tag="shift")
            inv_b = 1.0 / float(B)
            nc.vector.tensor_scalar_mul(mean[:], s12g[:, 0:8], inv_b)
            nc.vector.tensor_mul(scale[:], mean[:], mean[:])     # mean^2 (tmp)
            nc.vector.tensor_scalar_mul(var[:], s12g[:, 8:16], inv_b)
            nc.vector.tensor_sub(var[:], var[:], scale[:])       # var -= mean^2
            nc.vector.tensor_scalar_add(var[:], var[:], BN_EPS)
            nc.scalar.activation(var[:], var[:], AF.Sqrt)        # var = std
            nc.vector.reciprocal(scale[:], var[:])               # scale = 1/std
            nc.vector.tensor_mul(scale[:], scale[:], gam_t[:])   # *= gamma
            nc.vector.tensor_mul(shift[:], mean[:], scale[:])
            nc.vector.tensor_sub(shift[:], bet_t[:], shift[:])   # beta - m*s

            # --- BN apply + ReLU on ACT, then classifier matmuls ---
            o_ps = psB.tile([128, C], F32, tag="ops")
            for mb in range(8):
                r = cp.tile([128, 128], BF16, tag=f"rT{mb}", name=f"rT{mb}")
                nc.scalar.activation(
                    r[:], h_ps[mb // 4][:, (mb % 4) * 128:(mb % 4 + 1) * 128],
                    AF.Relu,
                    bias=shift[:, mb:mb + 1],
                    scale=scale[:, mb:mb + 1])
                nc.tensor.matmul(o_ps[:], lhsT=r[:], rhs=wclf_t[mb][:],
                                 start=(mb == 0), stop=False)
            # + b_clf via rank-1 ones matmul
            nc.tensor.matmul(o_ps[:], lhsT=on_t[:], rhs=bc_t[:],
                             start=False, stop=True)
            out_sb = cp.tile([128, C], F32, tag="outsb")
            nc.vector.tensor_copy(out_sb[:], o_ps[:])
            nc.sync.dma_start(logits[:], out_sb[:])

    nc.compile()
    return nc


def _get_program():
    global _PROGRAM
    if _PROGRAM is None:
        _PROGRAM = _build()
    return _PROGRAM


def _wrap_idx(lid):
    """[128, L] int16 local ids -> [128, (L/KC)*WC] wrapped dma_gather index
    tile: per chunk, linear j = k*128 + s reads lid[s, c*KC+k]; idx_j sits at
    [j%16, j//16]; the 16-partition block is replicated 8x."""
    L = lid.shape[1]
    cols = []
    for c in range(L // KC):
        lin = lid[:, c * KC:(c + 1) * KC].T.reshape(-1)  # j = k*128 + s
        cols.append(lin.reshape(-1, 16).T)               # [16, WC]
    wrapped = np.concatenate(cols, axis=1)               # [16, (L/KC)*WC]
    return np.ascontiguousarray(np.tile(wrapped, (8, 1)).astype(np.int16))


def _prep_in_maps(title, desc, t_len, d_len, emb, W_fc, b_fc, gamma, beta,
                  W_clf, b_clf):
    # sanitize: padded positions duplicate token 0 (keeps max exact; sum is
    # corrected on-device with the npad counts)
    def sanitize(tok, lens, L):
        tok = np.asarray(tok)
        valid = np.arange(L)[None, :] < np.asarray(lens)[:, None]
        return np.where(valid, tok, tok[:, :1]).astype(np.int32)

    title_s = sanitize(title, t_len, LT)
    desc_s = sanitize(desc, d_len, LD)
    t_len = np.asarray(t_len).astype(np.float64)
    d_len = np.asarray(d_len).astype(np.float64)
    scal = np.stack([
        1.0 / np.maximum(t_len, 1.0),
        1.0 / np.maximum(d_len, 1.0),
        np.zeros_like(t_len),
        np.zeros_like(d_len),
    ], axis=1).astype(np.float32)  # [B, 4]

    emb_bf = np.asarray(emb, dtype=np.float32).astype(NPBF)
    wfc = np.asarray(W_fc, dtype=np.float32).astype(NPBF)
    wclf = np.asarray(W_clf, dtype=np.float32).astype(NPBF)
    bclf = np.asarray(b_clf, dtype=np.float32).astype(NPBF).reshape(1, C)
    gamma_t = np.ascontiguousarray(
        np.asarray(gamma, dtype=np.float32).reshape(8, 128).T)
    beta_t = np.ascontiguousarray(
        np.asarray(beta, dtype=np.float32).reshape(8, 128).T)
    ident = np.eye(128, dtype=np.float32).astype(NPBF)
    ones1 = np.ones((1, 128), dtype=np.float32).astype(NPBF)

    # diag(-npad_chunk) per (field, chunk): npad_chunk[p] = # padded slots of
    # sample p among positions [c*KC, (c+1)*KC)
    def pad_counts(lens, L):
        cs = []
        for c in range(L // KC):
            a, b = c * KC, (c + 1) * KC
            cs.append(np.clip(b - np.maximum(lens, a), 0, KC))
        return cs  # list of [B]

    npc = pad_counts(t_len, LT) + pad_counts(d_len, LD)
    rng128 = np.arange(128)

    in_maps = []
    for i in range(N_CORES):
        sl = slice(i * PB, (i + 1) * PB)
        # per-core compacted vocab: <= 128*250 = 32000 unique rows, so int16
        # local ids always fit
        tok_all = np.concatenate([title_s[sl], desc_s[sl]], axis=1)  # [128,250]
        uniq, inv = np.unique(tok_all, return_inverse=True)
        lid = inv.reshape(PB, LT + LD).astype(np.int16)
        emb_loc = np.zeros((VLOC, D), dtype=NPBF)
        emb_loc[:len(uniq)] = emb_bf[uniq]
        dn = np.zeros((NCHUNKS * 128, 128), dtype=np.float32)
        for ci, cnt in enumerate(npc):
            dn[ci * 128 + rng128, rng128] = -cnt[sl].astype(np.float32)
        in_maps.append({
            "emb_loc": emb_loc,
            "t_widx": _wrap_idx(lid[:, :LT]),
            "d_widx": _wrap_idx(lid[:, LT:]),
            "scal": np.ascontiguousarray(scal[sl]),
            "wfc": wfc, "wclf": wclf, "bclf": bclf,
            "gamma_t": gamma_t, "beta_t": beta_t,
            "ident": ident, "ones1": ones1,
            "dnpad": dn.astype(NPBF),
        })
    return in_maps


def kernel(title, desc, t_len, d_len, emb, W_fc, b_fc, gamma, beta,
           W_clf, b_clf):
    nc = _get_program()
    in_maps = _prep_in_maps(title, desc, t_len, d_len, emb, W_fc, b_fc,
                            gamma, beta, W_clf, b_clf)
    res = bass_utils.run_bass_kernel_spmd(nc, in_maps,
                                          core_ids=list(range(N_CORES)))
    return np.concatenate([res.results[i]["logits"] for i in range(N_CORES)],
                          axis=0)
